# revision 5
# baseline (speedup 1.0000x reference)
"""Trainium2 Bass kernel for nn_AVGAE (3-layer GAT variational graph
autoencoder, N=4096) on 8 NeuronCores.

Sharding: 1D row partition of the N x N attention/score matrices — core k
owns output rows [512k, 512k+512). Small per-node features are all-gathered
between layers (AllGather over internal DRAM tiles).

Key algebraic restructuring (no elementwise transcendentals over N x N):
  exp(leaky_relu(f1_i + f2_j, a)) = max(A_i*B_j, C_i*D_j)
  with A=exp(f1), B=exp(f2), C=exp(a*f1), D=exp(a*f2)
so each N x N score tile is built with 3 vector ALU ops (outer-product
scalar mul, fused mul+max, mask mul), all bf16, then consumed directly by
the tensor engine as attention weights.  Softmax denominators come for free
as a ones-column in the attention rhs (exp(MASK_VAL) == 0 exactly in fp32,
so masked entries contribute 0 to both numerator and denominator, matching
the reference softmax).

All per-node "h" quantities of layers 1/2 are linear images of layer-0
attention output, so the layer-0 attention rhs carries
[h0@W1 | h0@W2 | per-layer score vectors | ones] and hidden itself is never
materialized.  Host precomputes the folded weight matrix; the device-side
first matmul is X_own @ Wbig.
"""

import numpy as np
import ml_dtypes

import concourse.bass as bass
import concourse.mybir as mybir
import concourse.tile as tile
from concourse import bacc
from concourse.bass import ts
from concourse.bass_utils import run_bass_kernel_spmd
from concourse.masks import make_identity

F32 = mybir.dt.float32
F32R = mybir.dt.float32r
BF16 = mybir.dt.bfloat16
AF = mybir.ActivationFunctionType
OP = mybir.AluOpType

N = 4096
INPUT_DIM = 512
H1 = 256
H2 = 64
ALPHA = 0.2
NCORES = 8
NB = N // NCORES          # 512 rows per core
IT = NB // 128            # 4 i-tiles per core
JT = N // 128             # 32 j-tiles

# G (layer-0 gathered rhs) column layout, width 136:
#   0:64 u1 | 64:128 u2 | 128 p1a | 129 p1b | 130 p2a | 131 p2b
#   | 132 ones | 133 B0 | 134 D0 | 135 pad
GW = 136
# G1 (layers 1+2 gathered rhs) column layout, width 136:
#   0:64 h1 | 64 ones | 65 B1 | 66 D1 | 67:131 h2 | 131 ones
#   | 132 B2 | 133 D2 | 134:136 pad
G1W = 136


def build_program():
    nc = bacc.Bacc("TRN2", target_bir_lowering=False, debug=False,
                   num_devices=NCORES)

    xt = nc.dram_tensor("xt", [INPUT_DIM, NB], F32R, kind="ExternalInput").ap()
    wbig = nc.dram_tensor("wbig", [INPUT_DIM, 134], F32R,
                          kind="ExternalInput").ap()
    maskT = nc.dram_tensor("maskT", [N, NB], BF16, kind="ExternalInput").ap()
    noise = nc.dram_tensor("noise", [NB, H2], F32, kind="ExternalInput").ap()
    apred = nc.dram_tensor("apred", [NB, N], F32, kind="ExternalOutput").ap()

    rg = [list(range(NCORES))]

    with tile.TileContext(nc) as tc, \
         tc.tile_pool(name="perm", bufs=1) as perm, \
         tc.tile_pool(name="gdram", bufs=1, space="DRAM") as gdram:

        # ---------- long-lived tiles ----------
        ident = perm.tile([128, 128], F32)
        make_identity(nc, ident)
        ones1 = perm.tile([1, 128], BF16)
        nc.vector.memset(ones1, 1.0)

        mask_sb = perm.tile([128, JT, NB], BF16)     # resident mask, 4 MB
        nc.sync.dma_start(out=mask_sb,
                          in_=maskT.rearrange("(t p) i -> p t i", p=128))

        bc0 = perm.tile([128, 2, NB], BF16)          # bcast A0 / C0
        bd0 = perm.tile([128, JT, 2], F32)           # f32 B0/D0 scalar cols
        bd12 = perm.tile([128, 2, JT, 2], F32)       # f32 B/D cols layers 1,2
        r0_sb = perm.tile([128, JT, GW], BF16)       # gathered layer-0 rhs
        bc12 = perm.tile([128, 4, NB], BF16)         # bcast A1,C1,A2,C2
        r1_sb = perm.tile([128, JT, G1W], BF16)      # gathered layer-1/2 rhs
        mean_sb = perm.tile([128, IT, H2], F32)
        noise_sb = perm.tile([128, IT, H2], F32)
        nc.sync.dma_start(out=noise_sb,
                          in_=noise.rearrange("(s p) c -> p s c", p=128))
        zl_sb = perm.tile([128, IT, H2], F32)        # noise * exp(logstd)
        zt_own = perm.tile([64, NB], F32R)
        ztb = perm.tile([64, NCORES, NB], F32R)

        g_in = gdram.tile([NB, GW], BF16)
        g_out = gdram.tile([N, GW], BF16, addr_space="Shared")
        g1_in = gdram.tile([NB, G1W], BF16)
        g1_out = gdram.tile([N, G1W], BF16, addr_space="Shared")
        ztg_in = gdram.tile([64, NB], F32R)
        ztg_out = gdram.tile([NCORES * 64, NB], F32R, addr_space="Shared")

        # ---------------- stage A: builder  R0_own = X_own @ Wbig ---------
        with tc.tile_pool(name="bld_sb", bufs=2) as bsb, \
             tc.tile_pool(name="bld_ps", bufs=2, space="PSUM") as bps:

            xt_sb = bsb.tile([128, 4, NB], F32R, tag="xt_sb")
            nc.sync.dma_start(out=xt_sb,
                              in_=xt.rearrange("(k p) n -> p k n", p=128))
            wb_sb = bsb.tile([128, 4, 134], F32R, tag="wb_sb")
            nc.sync.dma_start(out=wb_sb,
                              in_=wbig.rearrange("(k p) c -> p k c", p=128))

            a0row = bsb.tile([1, NB], BF16, tag="a0row")
            c0row = bsb.tile([1, NB], BF16, tag="c0row")

            for s in range(IT):
                psA = bps.tile([128, 134], F32, tag="psA")
                for k in range(4):
                    nc.tensor.matmul(psA, lhsT=xt_sb[:, k, ts(s, 128)],
                                     rhs=wb_sb[:, k, :],
                                     start=(k == 0), stop=(k == 3))
                gown = bsb.tile([128, GW], BF16, tag="gown")
                nc.scalar.activation(gown[:, 0:132], psA[:, 0:132], AF.Copy)
                nc.vector.memset(gown[:, 132:133], 1.0)
                nc.scalar.activation(gown[:, 133:134], psA[:, 133:134], AF.Exp)
                nc.scalar.activation(gown[:, 134:135], psA[:, 133:134], AF.Exp,
                                     scale=ALPHA)
                nc.vector.memset(gown[:, 135:136], 0.0)
                nc.sync.dma_start(out=g_in[ts(s, 128), :], in_=gown)

                # f10 column -> exp'd rows (A0 / C0)
                fcol = bsb.tile([128, 1], F32, tag="fcol")
                nc.scalar.activation(fcol, psA[:, 132:133], AF.Copy)
                psT = bps.tile([1, 128], F32, tag="psT")
                nc.tensor.transpose(psT, fcol, ident)
                nc.scalar.activation(a0row[0:1, ts(s, 128)], psT, AF.Exp)
                nc.scalar.activation(c0row[0:1, ts(s, 128)], psT, AF.Exp,
                                     scale=ALPHA)

            nc.gpsimd.collective_compute(
                "AllGather", OP.bypass, replica_groups=rg,
                ins=[g_in.opt()], outs=[g_out.opt()])

            for i, row in enumerate((a0row, c0row)):
                psB = bps.tile([128, NB], F32, tag="psB")
                nc.tensor.matmul(psB, lhsT=ones1, rhs=row, start=True,
                                 stop=True)
                nc.scalar.activation(bc0[:, i, :], psB, AF.Copy)

            nc.sync.dma_start(out=r0_sb,
                              in_=g_out.rearrange("(t p) c -> p t c", p=128))
            nc.vector.tensor_copy(bd0, r0_sb[:, :, 133:135])

        # ---------------- stage C: layer-0 attention pass ------------------
        with tc.tile_pool(name="p0_ps", bufs=1, space="PSUM") as p0ps, \
             tc.tile_pool(name="p0_v", bufs=3) as vp:

            ps0 = [p0ps.tile([128, 133], F32, tag=f"ps0_{s}",
                             name=f"ps0_{s}") for s in range(IT)]
            for t in range(JT):
                t2 = vp.tile([128, NB], BF16, tag="t2")
                nc.vector.tensor_scalar_mul(t2, bc0[:, 1, :],
                                            bd0[:, t, 1:2])
                t3 = vp.tile([128, NB], BF16, tag="t3")
                nc.vector.scalar_tensor_tensor(
                    t3, in0=bc0[:, 0, :], scalar=bd0[:, t, 0:1],
                    in1=t2, op0=OP.mult, op1=OP.max)
                pt = vp.tile([128, NB], BF16, tag="pt")
                nc.vector.tensor_tensor(pt, t3, mask_sb[:, t, :], op=OP.mult)
                for s in range(IT):
                    nc.tensor.matmul(ps0[s], lhsT=pt[:, ts(s, 128)],
                                     rhs=r0_sb[:, t, 0:133],
                                     start=(t == 0), stop=(t == JT - 1))

            # ---------------- stage D: normalize + build G1 ----------------
            with tc.tile_pool(name="d_sb", bufs=2) as dsb, \
                 tc.tile_pool(name="d_ps", bufs=1, space="PSUM") as dps:

                rows12 = dsb.tile([1, 4, NB], BF16, tag="rows12", bufs=1)

                for s in range(IT):
                    r0c = dsb.tile([128, 1], F32, tag="r0c")
                    nc.vector.reciprocal(r0c, ps0[s][:, 132:133])
                    r0a = dsb.tile([128, 1], F32, tag="r0a")
                    nc.vector.tensor_scalar_mul(r0a, r0c, ALPHA)

                    g1own = dsb.tile([128, G1W], BF16, tag="g1own")
                    nc.scalar.activation(g1own[:, 0:64], ps0[s][:, 0:64],
                                         AF.Copy, scale=r0c)
                    nc.vector.memset(g1own[:, 64:65], 1.0)
                    nc.scalar.activation(g1own[:, 65:66], ps0[s][:, 129:130],
                                         AF.Exp, scale=r0c)
                    nc.scalar.activation(g1own[:, 66:67], ps0[s][:, 129:130],
                                         AF.Exp, scale=r0a)
                    nc.scalar.activation(g1own[:, 67:131], ps0[s][:, 64:128],
                                         AF.Copy, scale=r0c)
                    nc.vector.memset(g1own[:, 131:132], 1.0)
                    nc.scalar.activation(g1own[:, 132:133], ps0[s][:, 131:132],
                                         AF.Exp, scale=r0c)
                    nc.scalar.activation(g1own[:, 133:134], ps0[s][:, 131:132],
                                         AF.Exp, scale=r0a)
                    nc.vector.memset(g1own[:, 134:136], 0.0)
                    nc.sync.dma_start(out=g1_in[ts(s, 128), :], in_=g1own)

                    # f1' (col 128) and f1'' (col 130) -> exp'd rows
                    for li, col in ((0, 128), (2, 130)):
                        fcl = dsb.tile([128, 1], F32, tag="fcl")
                        nc.scalar.activation(fcl, ps0[s][:, col:col + 1],
                                             AF.Copy, scale=r0c)
                        psT2 = dps.tile([1, 128], F32, tag="psT2")
                        nc.tensor.transpose(psT2, fcl, ident)
                        nc.scalar.activation(rows12[0:1, li, ts(s, 128)],
                                             psT2, AF.Exp)
                        nc.scalar.activation(rows12[0:1, li + 1, ts(s, 128)],
                                             psT2, AF.Exp, scale=ALPHA)

                nc.gpsimd.collective_compute(
                    "AllGather", OP.bypass, replica_groups=rg,
                    ins=[g1_in.opt()], outs=[g1_out.opt()])

                for i in range(4):
                    psB2 = dps.tile([128, NB], F32, tag="psB2")
                    nc.tensor.matmul(psB2, lhsT=ones1,
                                     rhs=rows12[0:1, i, :], start=True,
                                     stop=True)
                    nc.scalar.activation(bc12[:, i, :], psB2, AF.Copy)

                nc.sync.dma_start(
                    out=r1_sb, in_=g1_out.rearrange("(t p) c -> p t c", p=128))
                nc.vector.tensor_copy(bd12[:, 0, :, :], r1_sb[:, :, 65:67])
                nc.vector.tensor_copy(bd12[:, 1, :, :], r1_sb[:, :, 132:134])

        # ---------------- stage E: layer-1 and layer-2 passes --------------
        # layer 1: rhs = G1[:, 0:65], B=65, D=66, bc rows 0/1
        # layer 2: rhs = G1[:, 67:132], B=132, D=133, bc rows 2/3
        for layer in (1, 2):
            c0 = 0 if layer == 1 else 67
            cb = 65 if layer == 1 else 132
            with tc.tile_pool(name=f"p{layer}_ps", bufs=1,
                              space="PSUM") as pps, \
                 tc.tile_pool(name=f"p{layer}_v", bufs=3) as vpl, \
                 tc.tile_pool(name=f"p{layer}_s", bufs=2) as spl:
                psl = [pps.tile([128, 65], F32, tag=f"ps{layer}_{s}",
                                name=f"ps{layer}_{s}") for s in range(IT)]
                for t in range(JT):
                    t2 = vpl.tile([128, NB], BF16, tag="t2")
                    nc.vector.tensor_scalar_mul(
                        t2, bc12[:, 2 * layer - 1, :],
                        bd12[:, layer - 1, t, 1:2])
                    t3 = vpl.tile([128, NB], BF16, tag="t3")
                    nc.vector.scalar_tensor_tensor(
                        t3, in0=bc12[:, 2 * layer - 2, :],
                        scalar=bd12[:, layer - 1, t, 0:1],
                        in1=t2, op0=OP.mult, op1=OP.max)
                    pt = vpl.tile([128, NB], BF16, tag="pt")
                    nc.vector.tensor_tensor(pt, t3, mask_sb[:, t, :],
                                            op=OP.mult)
                    for s in range(IT):
                        nc.tensor.matmul(
                            psl[s], lhsT=pt[:, ts(s, 128)],
                            rhs=r1_sb[:, t, c0:c0 + 65],
                            start=(t == 0), stop=(t == JT - 1))

                for s in range(IT):
                    rc = spl.tile([128, 1], F32, tag="rc")
                    nc.vector.reciprocal(rc, psl[s][:, 64:65])
                    if layer == 1:
                        nc.scalar.activation(mean_sb[:, s, :],
                                             psl[s][:, 0:64], AF.Copy,
                                             scale=rc)
                    else:
                        els = spl.tile([128, H2], F32, tag="els")
                        nc.scalar.activation(els, psl[s][:, 0:64],
                                             AF.Exp, scale=rc)
                        nc.vector.tensor_tensor(zl_sb[:, s, :], els,
                                                noise_sb[:, s, :],
                                                op=OP.mult)

        # ---------------- stage F: Z assembly, transpose, gather -----------
        with tc.tile_pool(name="f_sb", bufs=2) as fsb, \
             tc.tile_pool(name="f_ps", bufs=2, space="PSUM") as fps:

            for s in range(IT):
                z_s = fsb.tile([128, H2], F32, tag="z_s")
                nc.vector.tensor_tensor(z_s, zl_sb[:, s, :],
                                        mean_sb[:, s, :], op=OP.add)
                psZ = fps.tile([64, 128], F32, tag="psZ")
                nc.tensor.transpose(psZ, z_s, ident)
                nc.scalar.activation(zt_own[:, ts(s, 128)], psZ, AF.Copy)

            nc.sync.dma_start(out=ztg_in, in_=zt_own)
            nc.gpsimd.collective_compute(
                "AllGather", OP.bypass, replica_groups=rg,
                ins=[ztg_in.opt()], outs=[ztg_out.opt()])
            nc.sync.dma_start(
                out=ztb, in_=ztg_out.rearrange("(b p) i -> p b i", p=64))

        # ---------------- stage G: decoder sigmoid(Z @ Z^T) ----------------
        with tc.tile_pool(name="dec_ps", bufs=4, space="PSUM") as decps, \
             tc.tile_pool(name="dec_sb", bufs=4) as decsb:
            for s in range(IT):
                for b in range(NCORES):
                    psD = decps.tile([128, NB], F32, tag="psD")
                    nc.tensor.matmul(psD, lhsT=zt_own[:, ts(s, 128)],
                                     rhs=ztb[:, b, :], start=True,
                                     stop=True)
                    osb = decsb.tile([128, NB], F32, tag="osb")
                    nc.scalar.activation(osb, psD, AF.Sigmoid)
                    nc.sync.dma_start(
                        out=apred[ts(s, 128), ts(b, NB)], in_=osb)

    nc.compile()
    return nc


_program = None


def _get_program():
    global _program
    if _program is None:
        _program = build_program()
    return _program


def kernel(X, adj, noise, W0, a0, W1, a1, W2, a2, _trace=False):
    X = np.asarray(X, dtype=np.float32)
    adj = np.asarray(adj)
    noise = np.asarray(noise, dtype=np.float32)
    W0 = np.asarray(W0, dtype=np.float32)
    a0 = np.asarray(a0, dtype=np.float32)
    W1 = np.asarray(W1, dtype=np.float32)
    a1 = np.asarray(a1, dtype=np.float32)
    W2 = np.asarray(W2, dtype=np.float32)
    a2 = np.asarray(a2, dtype=np.float32)

    # folded weight matrix [512, 134]
    u1 = W0 @ W1
    u2 = W0 @ W2
    wbig = np.concatenate([
        u1, u2,
        u1 @ a1[:H2], u1 @ a1[H2:],
        u2 @ a2[:H2], u2 @ a2[H2:],
        W0 @ a0[:H1], W0 @ a0[H1:],
    ], axis=1).astype(np.float32)

    maskT = adj.astype(ml_dtypes.bfloat16).T  # 0/1, exact in bf16

    in_maps = []
    for k in range(NCORES):
        sl = slice(k * NB, (k + 1) * NB)
        in_maps.append({
            "xt": np.ascontiguousarray(X[sl].T),
            "wbig": wbig,
            "maskT": np.ascontiguousarray(maskT[:, sl]),
            "noise": noise[sl],
        })

    nc = _get_program()
    res = run_bass_kernel_spmd(nc, in_maps, core_ids=list(range(NCORES)),
                               trace=_trace)
    out = np.concatenate([res.results[k]["apred"] for k in range(NCORES)],
                         axis=0)
    if _trace:
        kernel.last_results = res
    return out
